# revision 39
# baseline (speedup 1.0000x reference)
"""nn_Attention_3d — 3D windowed attention with decomposed relative position
biases, on 8 Trainium2 NeuronCores via Bass/Tile.

Sharding: data-parallel over the window dim B (32 windows -> 4 per core).
Weights / rel-pos tables replicated.

Math notes (vs reference):
  - softmax max-subtraction dropped: logits are O(1) (inputs ~N(0,1), weights
    ~0.02 scale), exp cannot overflow; softmax is shift-invariant so the
    result is identical.
  - q-side qkv bias kept (it feeds the rel-pos terms and the k-dot); k-side
    bias dropped (adds a per-row constant to logits -> softmax invariant).
  - v-side bias folded into the proj bias on host: (o + bv) @ Wp^T + bp
    = o @ Wp^T + (bv @ Wp^T + bp).
  - attn scale folded into Wk on host; lr folded into the rel tables.

Device pipeline per window (N=512 tokens, C=768, 12 heads, hd=64):
  qkvT   : per-head-pair column-reordered QKV matmul, bf16, PSUM->SBUF evac
  v      : token-major V matmul with an appended ones column per head
           (the PV matmul then yields the softmax denominator for free)
  rel    : per (head, axis, coord-chunk) tiny matmuls against host-built
           rel tables -> relsel^T (24 x 512) + zero padding to 96 rows
  logits : logits^T[m,n] = k^T.T @ q^T  (+ G96.T @ relstage accumulate)
  exp    : ScalarE Exp, PSUM fp32 -> SBUF bf16 (p^T tiles)
  PV     : out[n, head*65+c] accumulated over m K-tiles; col 64 = sum = s[n]
  scale  : out *= 1/s per (row, head) on VectorE
  trans  : TensorE transpose to channel-major for the proj matmul
  proj   : out @ Wp^T + bias via K=1 ones-row matmul; PSUM -> DRAM DMA
"""

import sys

sys.path.insert(0, "/opt/trn_rl_repo")

import numpy as np
import ml_dtypes

import concourse.bass as bass
import concourse.tile as tile
from concourse import bacc, mybir
from concourse.bass_utils import run_bass_kernel_spmd

F32 = mybir.dt.float32
BF16 = mybir.dt.bfloat16
AF = mybir.ActivationFunctionType

B, HH, WW, DD, C = 32, 8, 8, 8, 768
NH, HD = 12, 64
N = HH * WW * DD  # 512
NCORES = 8
WPC = B // NCORES  # 4 windows per core
BF = ml_dtypes.bfloat16


def _rel_table(rel_pos, n):
    idx = np.arange(n)[:, None] - np.arange(n)[None, :] + (n - 1)
    return rel_pos[idx]  # (n, n, hd)


def _bcast(ap, n):
    """Append a stride-0 free dim of size n to an AP."""
    return bass.AP(tensor=ap.tensor, offset=ap.offset, ap=[*ap.ap, [0, n]])


def _body(tc, out_ap, xT_ap, wqk_ap, wv_ap, wp_ap, bq_ap, bp_ap, rt_ap, g96_ap, id_ap):
    nc = tc.nc
    import contextlib

    ctx = contextlib.ExitStack()
    with ctx:
        singles = ctx.enter_context(tc.tile_pool(name="singles", bufs=1))
        xp = ctx.enter_context(tc.tile_pool(name="xp", bufs=2))
        qkp = ctx.enter_context(tc.tile_pool(name="qkp", bufs=8))
        vhp = ctx.enter_context(tc.tile_pool(name="vhp", bufs=6))
        relp = ctx.enter_context(tc.tile_pool(name="relp", bufs=3))
        ptp = ctx.enter_context(tc.tile_pool(name="ptp", bufs=26))
        osbp = ctx.enter_context(tc.tile_pool(name="osbp", bufs=6))
        otTp = ctx.enter_context(tc.tile_pool(name="otTp", bufs=8))
        rcpp = ctx.enter_context(tc.tile_pool(name="rcpp", bufs=6))
        foutp = ctx.enter_context(tc.tile_pool(name="foutp", bufs=4))
        psA = ctx.enter_context(tc.tile_pool(name="psA", bufs=2, space="PSUM"))
        psB = ctx.enter_context(tc.tile_pool(name="psB", bufs=4, space="PSUM"))

        # --- load constants (split per K-tile so compute starts early) ---
        wqk_sb = singles.tile([128, 6, 12, 128], BF16, tag="wqk")
        wv_sb = singles.tile([128, 6, 768], BF16, tag="wv")
        wp_sb = singles.tile([128, 6, 768], BF16, tag="wp")
        for ct in range(6):
            nc.sync.dma_start(out=wqk_sb[:, ct], in_=wqk_ap[:, ct])
        for ct in range(6):
            nc.sync.dma_start(out=wv_sb[:, ct], in_=wv_ap[:, ct])
            nc.sync.dma_start(out=wp_sb[:, ct], in_=wp_ap[:, ct])
        bq_sb = singles.tile([128, 6], F32, tag="bq")
        nc.sync.dma_start(out=bq_sb, in_=bq_ap)
        bp_sb = singles.tile([1, 768], BF16, tag="bp")
        nc.sync.dma_start(out=bp_sb, in_=bp_ap)
        rt_sb = singles.tile([128, 3, 8, 64], BF16, tag="rt")
        nc.sync.dma_start(out=rt_sb, in_=rt_ap)
        # persistent zero-padded K=128 k-operands, one per head; rows outside
        # the head's 64 channels stay zero forever (memset once here)
        kpad = []
        for h in range(12):
            t = singles.tile([128, N], BF16, tag=f"kpad{h}")
            nc.vector.memset(t, 0.0)
            kpad.append(t)
        g96_sb = singles.tile([128, N], BF16, tag="g96")
        nc.sync.dma_start(out=g96_sb, in_=g96_ap)
        id_sb = singles.tile([128, 128], BF16, tag="ident")
        nc.sync.dma_start(out=id_sb, in_=id_ap)
        ones1 = singles.tile([1, 128], BF16, tag="ones1")
        nc.vector.memset(ones1, 1.0)

        for w in range(WPC):
            xsb = xp.tile([128, 6, N], BF16, tag="x")
            for ct in range(6):
                nc.sync.dma_start(out=xsb[:, ct], in_=xT_ap[w, :, ct])

            # --- QKV projection (q tiles 0-5, k tiles 6-11; head-pair cols) ---
            qksb = []
            for ot in range(12):
                ps = psB.tile([128, N], F32, tag="ps1")
                for ct in range(6):
                    nc.tensor.matmul(
                        ps,
                        lhsT=wqk_sb[:, ct, ot, :],
                        rhs=xsb[:, ct, :],
                        start=(ct == 0),
                        stop=(ct == 5),
                    )
                if ot < 6:
                    t = qkp.tile([128, N], BF16, tag="qk")
                    if ot % 2 == 0:
                        nc.vector.tensor_scalar_add(
                            out=t, in0=ps, scalar1=bq_sb[:, ot : ot + 1]
                        )
                    else:
                        nc.scalar.activation(
                            out=t, in_=ps, func=AF.Identity,
                            bias=bq_sb[:, ot : ot + 1],
                        )
                    qksb.append(t)
                else:
                    hA, hB = 2 * (ot - 6), 2 * (ot - 6) + 1
                    if ot % 2 == 0:
                        nc.vector.tensor_copy(out=kpad[hA][0:64], in_=ps[0:64])
                        nc.scalar.copy(out=kpad[hB][64:128], in_=ps[64:128])
                    else:
                        nc.scalar.copy(out=kpad[hA][0:64], in_=ps[0:64])
                        nc.vector.tensor_copy(out=kpad[hB][64:128], in_=ps[64:128])

            # --- V (token-major) with ones column per head ---
            vhat = []
            for mt in range(4):
                vt = vhp.tile([128, NH * 65], BF16, tag="vh")
                vt3 = vt.rearrange("p (h e) -> p h e", e=65)
                nc.vector.memset(vt3[:, :, 64], 1.0)
                for half in range(2):
                    ps = psB.tile([128, 384], F32, tag="ps1")
                    for ct in range(6):
                        nc.tensor.matmul(
                            ps,
                            lhsT=xsb[:, ct, mt * 128 : (mt + 1) * 128],
                            rhs=wv_sb[:, ct, 384 * half : 384 * (half + 1)],
                            start=(ct == 0),
                            stop=(ct == 5),
                        )
                    nc.vector.tensor_copy(
                        out=vt3[:, 6 * half : 6 * (half + 1), 0:64],
                        in_=ps.rearrange("p (h c) -> p h c", c=64),
                    )
                vhat.append(vt)

            # --- attention: rel bias, logits^T, exp ---
            pts = []
            for hp in range(6):
                qt = qksb[hp]
                qt4 = qt.rearrange("p (u v z) -> p u v z", v=8, z=8)

                # rel, pair-batched: per axis one (64,512) PSUM; rows 0-31 =
                # head 2hp, rows 32-63 = head 2hp+1 (block-diag rt tables).
                # h-axis stored in std col order; d stored (z,u,v)-major;
                # w stored (v,u,z)-major; un-permuted during the casts.
                rpsa = []
                for a in range(3):
                    rps = psB.tile([64, N], F32, tag="ps1")
                    for j in range(8):
                        if a == 0:  # h: coord = n // 64; dense chunk
                            rhs = qt[:, 64 * j : 64 * (j + 1)]
                        elif a == 1:  # d: coord = n % 8; stream (u, v)
                            rhs = qt4[:, :, :, j]
                        else:  # w: coord = (n // 8) % 8; stream (u, z)
                            rhs = qt4[:, :, j, :]
                        nc.tensor.matmul(
                            rps[:, 64 * j : 64 * (j + 1)],
                            lhsT=rt_sb[:, a, j, :], rhs=rhs,
                            start=(j == 0), stop=(j == 7),
                            skip_group_check=True,
                        )
                    rpsa.append(rps)

                for v in range(2):
                    h = 2 * hp + v
                    s = 32 * v
                    rstage = relp.tile([128, N], BF16, tag="rel")
                    # stored->token un-permutes (token n = 64u + 8v + z)
                    rps_d = rpsa[1][s : s + 32].rearrange(
                        "p (z u v) -> p u v z", u=8, v=8
                    )
                    rps_w = rpsa[2][s : s + 32].rearrange(
                        "p (v u z) -> p u v z", u=8, z=8
                    )
                    e1, e2 = (
                        (nc.vector, nc.scalar) if v == 0 else (nc.scalar, nc.vector)
                    )
                    c1 = e1.tensor_copy if e1 is nc.vector else e1.copy
                    c2 = e2.tensor_copy if e2 is nc.vector else e2.copy
                    c1(out=rstage[0:32], in_=rpsa[0][s : s + 32])
                    c2(out=rstage[32:64], in_=rps_d)
                    c1(out=rstage[64:96], in_=rps_w)
                    nc.gpsimd.memset(rstage[96:128], 0.0)

                    pth = []
                    for mtp in range(2):
                        lps = psA.tile([128, 1024], F32, tag="ps2")
                        for i in range(2):
                            mt = 2 * mtp + i
                            sl = lps[:, 512 * i : 512 * (i + 1)]
                            nc.tensor.matmul(
                                sl,
                                lhsT=kpad[h][:, mt * 128 : (mt + 1) * 128],
                                rhs=qt,
                                start=True,
                                stop=False,
                            )
                            nc.tensor.matmul(
                                sl,
                                lhsT=g96_sb[:, mt * 128 : (mt + 1) * 128],
                                rhs=rstage,
                                start=False,
                                stop=True,
                            )
                        pt_t = ptp.tile([128, 1024], BF16, tag="pt")
                        nc.scalar.activation(out=pt_t, in_=lps, func=AF.Exp)
                        pth.append(pt_t)
                    pts.append(pth)

            # --- PV + 1/s scaling, per token tile ---
            outsb = []
            for nt in range(4):
                halves = []
                for half in range(2):
                    pv = psB.tile([128, 6 * 65], F32, tag="ps1")
                    pv3 = pv.rearrange("p (h e) -> p h e", e=65)
                    for hh in range(6):
                        h = 6 * half + hh
                        for kt_i in range(4):
                            mtp, i = divmod(kt_i, 2)
                            lhsT = pts[h][mtp][
                                :, 512 * i + 128 * nt : 512 * i + 128 * (nt + 1)
                            ]
                            rhs = vhat[kt_i].rearrange("p (g e) -> p g e", e=65)[
                                :, h, :
                            ]
                            nc.tensor.matmul(
                                pv3[:, hh, :], lhsT=lhsT, rhs=rhs,
                                start=(kt_i == 0 and hh == 0),
                                stop=(kt_i == 3),
                                skip_group_check=True,
                            )
                    halves.append(pv)
                rc = rcpp.tile([128, 12], F32, tag="rcp")
                for half in range(2):
                    pv3 = halves[half].rearrange("p (h e) -> p h e", e=65)
                    nc.vector.reciprocal(
                        out=rc[:, 6 * half : 6 * (half + 1)], in_=pv3[:, :, 64]
                    )
                ot_sb = osbp.tile([128, 768], BF16, tag="osb")
                ot3 = ot_sb.rearrange("p (h c) -> p h c", c=64)
                for half in range(2):
                    pv3 = halves[half].rearrange("p (h e) -> p h e", e=65)
                    nc.vector.tensor_mul(
                        out=ot3[:, 6 * half : 6 * (half + 1), :],
                        in0=pv3[:, :, 0:64],
                        in1=_bcast(rc[:, 6 * half : 6 * (half + 1)], 64),
                    )
                outsb.append(ot_sb)

            # --- transpose to channel-major ---
            outT = []
            for ct in range(6):
                tp = psB.tile([128, N], BF16, tag="ps1")
                for nt in range(4):
                    nc.tensor.transpose(
                        out=tp[:, 128 * nt : 128 * (nt + 1)],
                        in_=outsb[nt][:, 128 * ct : 128 * (ct + 1)],
                        identity=id_sb,
                    )
                tT = otTp.tile([128, N], BF16, tag="otT")
                nc.vector.tensor_copy(out=tT, in_=tp)
                outT.append(tT)

            # --- proj + bias, DMA out (384-col halves in separate banks) ---
            for nt in range(4):
                pr = psA.tile([128, 1024], F32, tag="ps2")
                for half in range(2):
                    sl = pr[:, 512 * half : 512 * half + 384]
                    for ct in range(6):
                        nc.tensor.matmul(
                            sl,
                            lhsT=outT[ct][:, 128 * nt : 128 * (nt + 1)],
                            rhs=wp_sb[:, ct, 384 * half : 384 * (half + 1)],
                            start=(ct == 0),
                            stop=False,
                        )
                    nc.tensor.matmul(
                        sl,
                        lhsT=ones1,
                        rhs=bp_sb[:, 384 * half : 384 * (half + 1)],
                        start=False,
                        stop=True,
                    )
                fo = foutp.tile([128, 768], F32, tag="fo")
                pr_v = pr.rearrange("p (h c) -> p h c", c=512)[:, :, 0:384]
                fo_v = fo.rearrange("p (h c) -> p h c", c=384)
                if nt % 2 == 0:
                    nc.vector.tensor_copy(out=fo_v, in_=pr_v)
                else:
                    nc.scalar.copy(out=fo_v, in_=pr_v)
                nc.sync.dma_start(
                    out=out_ap[w, 128 * nt : 128 * (nt + 1), :], in_=fo,
                )


_CACHED = None


def build_module():
    global _CACHED
    if _CACHED is not None:
        return _CACHED
    nc = bacc.Bacc("TRN2", target_bir_lowering=False, debug=False,
                   num_devices=NCORES)
    xT = nc.dram_tensor("xT", [WPC, 128, 6, N], BF16, kind="ExternalInput").ap()
    wqk = nc.dram_tensor("wqk", [128, 6, 12, 128], BF16, kind="ExternalInput").ap()
    wv = nc.dram_tensor("wv", [128, 6, 768], BF16, kind="ExternalInput").ap()
    wp = nc.dram_tensor("wp", [128, 6, 768], BF16, kind="ExternalInput").ap()
    bq = nc.dram_tensor("bq", [128, 6], F32, kind="ExternalInput").ap()
    bp = nc.dram_tensor("bp", [1, 768], BF16, kind="ExternalInput").ap()
    rt = nc.dram_tensor("rt", [128, 3, 8, 64], BF16, kind="ExternalInput").ap()
    g96 = nc.dram_tensor("g96", [128, N], BF16, kind="ExternalInput").ap()
    idm = nc.dram_tensor("idm", [128, 128], BF16, kind="ExternalInput").ap()
    out = nc.dram_tensor("out", [WPC, N, C], F32, kind="ExternalOutput").ap()
    with tile.TileContext(nc) as tc:
        _body(tc, out, xT, wqk, wv, wp, bq, bp, rt, g96, idm)
    nc.compile()
    _CACHED = nc
    return nc


def prep_inputs(inputs):
    """Host-side layout/folding. Returns the shared in_map (no xT) and the
    per-core xT arrays."""
    x = np.asarray(inputs["x"], np.float32)
    qkv_w = np.asarray(inputs["qkv_w"], np.float32)
    qkv_b = np.asarray(inputs["qkv_b"], np.float32)
    proj_w = np.asarray(inputs["proj_w"], np.float32)
    proj_b = np.asarray(inputs["proj_b"], np.float32)
    lr = float(np.asarray(inputs["lr"]))
    scale = HD ** -0.5

    Wq, Wk, Wv = qkv_w[0:C], qkv_w[C : 2 * C], qkv_w[2 * C : 3 * C]
    bq_, bv_ = qkv_b[0:C], qkv_b[2 * C : 3 * C]

    # wqk: [p, ct, ot, mc]; o-tiles 0-5 = q head-pairs, 6-11 = scale*Wk pairs
    wqk = np.zeros((128, 6, 12, 128), np.float32)
    for ot in range(12):
        for half in range(2):
            if ot < 6:
                head = 2 * ot + half
                Wsrc = Wq[head * 64 : (head + 1) * 64]
            else:
                head = 2 * (ot - 6) + half
                Wsrc = Wk[head * 64 : (head + 1) * 64] * scale
            wt = Wsrc.T.reshape(6, 128, 64).transpose(1, 0, 2)  # (p, ct, 64)
            wqk[:, :, ot, 64 * half : 64 * (half + 1)] = wt

    wv = Wv.T.reshape(6, 128, C).transpose(1, 0, 2)  # (p, ct, co)
    wp = proj_w.T.reshape(6, 128, C).transpose(1, 0, 2)
    bq_t = bq_.reshape(6, 128).T.copy()  # (128, 6), head-pair order matches
    bp1 = (proj_b + bv_ @ proj_w.T).reshape(1, C)

    # region order: h, d, w (matches the on-device rel layout)
    tabs = [
        _rel_table(np.asarray(inputs["rel_pos_h"], np.float32), 8) * lr,
        _rel_table(np.asarray(inputs["rel_pos_d"], np.float32), 8) * lr,
        _rel_table(np.asarray(inputs["rel_pos_w"], np.float32), 8) * lr,
    ]
    # rt: block-diag pair tables — lhsT cols 0-31 give head A's 32-row rel
    # block (contracting its q rows 0-63), cols 32-63 head B's (rows 64-127)
    rt = np.zeros((128, 3, 8, 64), np.float32)
    for a in range(3):
        t = tabs[a].transpose(2, 0, 1)  # (64c, 8j, 8k)
        rt[0:64, a, :, 0:8] = t
        rt[64:128, a, :, 32:40] = t

    m = np.arange(N)
    g96 = np.zeros((128, N), np.float32)
    coords = [m // 64, m % 8, (m // 8) % 8]
    for a in range(3):
        for k in range(8):
            g96[32 * a + k] = coords[a] == k

    xall = (
        x.reshape(B, N, 6, 128).transpose(0, 3, 2, 1).astype(BF)
    )  # (B, p, ct, n)

    shared = {
        "wqk": wqk.astype(BF),
        "wv": wv.astype(BF),
        "wp": wp.astype(BF),
        "bq": bq_t.astype(np.float32),
        "bp": bp1.astype(BF),
        "rt": rt.astype(BF),
        "g96": g96.astype(BF),
        "idm": np.eye(128, dtype=np.float32).astype(BF),
    }
    xT_cores = [xall[WPC * i : WPC * (i + 1)] for i in range(NCORES)]
    return shared, xT_cores


def assemble_output(results):
    outs = [np.asarray(r["out"], np.float32) for r in results]
    full = np.concatenate(outs, axis=0)  # (32, 512, 768)
    return full.reshape(B, HH, WW, DD, C)


def kernel(x, qkv_w, qkv_b, proj_w, proj_b, rel_pos_h, rel_pos_w, rel_pos_d, lr,
           _trace=False):
    nc = build_module()
    shared, xT_cores = prep_inputs(dict(
        x=x, qkv_w=qkv_w, qkv_b=qkv_b, proj_w=proj_w, proj_b=proj_b,
        rel_pos_h=rel_pos_h, rel_pos_w=rel_pos_w, rel_pos_d=rel_pos_d, lr=lr,
    ))
    in_maps = [{**shared, "xT": xT_cores[i]} for i in range(NCORES)]
    res = run_bass_kernel_spmd(nc, in_maps, list(range(NCORES)), trace=_trace)
    out = assemble_output(res.results)
    if _trace:
        kernel.last_exec_time_ns = res.exec_time_ns
        kernel.last_profile = res
    return out


# revision 41
# speedup vs baseline: 1.0106x; 1.0106x over previous
"""nn_Attention_3d — 3D windowed attention with decomposed relative position
biases, on 8 Trainium2 NeuronCores via Bass/Tile.

Sharding: data-parallel over the window dim B (32 windows -> 4 per core).
Weights / rel-pos tables replicated.

Math notes (vs reference):
  - softmax max-subtraction dropped: logits are O(1) (inputs ~N(0,1), weights
    ~0.02 scale), exp cannot overflow; softmax is shift-invariant so the
    result is identical.
  - q-side qkv bias kept (it feeds the rel-pos terms and the k-dot); k-side
    bias dropped (adds a per-row constant to logits -> softmax invariant).
  - v-side bias folded into the proj bias on host: (o + bv) @ Wp^T + bp
    = o @ Wp^T + (bv @ Wp^T + bp).
  - attn scale folded into Wk on host; lr folded into the rel tables.

Device pipeline per window (N=512 tokens, C=768, 12 heads, hd=64):
  qkvT   : per-head-pair column-reordered QKV matmul, bf16, PSUM->SBUF evac
  v      : token-major V matmul with an appended ones column per head
           (the PV matmul then yields the softmax denominator for free)
  rel    : per (head, axis, coord-chunk) tiny matmuls against host-built
           rel tables -> relsel^T (24 x 512) + zero padding to 96 rows
  logits : logits^T[m,n] = k^T.T @ q^T  (+ G96.T @ relstage accumulate)
  exp    : ScalarE Exp, PSUM fp32 -> SBUF bf16 (p^T tiles)
  PV     : out[n, head*65+c] accumulated over m K-tiles; col 64 = sum = s[n]
  scale  : out *= 1/s per (row, head) on VectorE
  trans  : TensorE transpose to channel-major for the proj matmul
  proj   : out @ Wp^T + bias via K=1 ones-row matmul; PSUM -> DRAM DMA
"""

import sys

sys.path.insert(0, "/opt/trn_rl_repo")

import numpy as np
import ml_dtypes

import concourse.bass as bass
import concourse.tile as tile
from concourse import bacc, mybir
from concourse.bass_utils import run_bass_kernel_spmd

F32 = mybir.dt.float32
BF16 = mybir.dt.bfloat16
AF = mybir.ActivationFunctionType

B, HH, WW, DD, C = 32, 8, 8, 8, 768
NH, HD = 12, 64
N = HH * WW * DD  # 512
NCORES = 8
WPC = B // NCORES  # 4 windows per core
BF = ml_dtypes.bfloat16


def _rel_table(rel_pos, n):
    idx = np.arange(n)[:, None] - np.arange(n)[None, :] + (n - 1)
    return rel_pos[idx]  # (n, n, hd)


def _bcast(ap, n):
    """Append a stride-0 free dim of size n to an AP."""
    return bass.AP(tensor=ap.tensor, offset=ap.offset, ap=[*ap.ap, [0, n]])


def _body(tc, out_ap, xT_ap, wqk_ap, wv_ap, wp_ap, bq_ap, bp_ap, rt_ap, g96_ap, id_ap):
    nc = tc.nc
    import contextlib

    ctx = contextlib.ExitStack()
    with ctx:
        singles = ctx.enter_context(tc.tile_pool(name="singles", bufs=1))
        xp = ctx.enter_context(tc.tile_pool(name="xp", bufs=2))
        qkp = ctx.enter_context(tc.tile_pool(name="qkp", bufs=8))
        vhp = ctx.enter_context(tc.tile_pool(name="vhp", bufs=6))
        relp = ctx.enter_context(tc.tile_pool(name="relp", bufs=3))
        ptp = ctx.enter_context(tc.tile_pool(name="ptp", bufs=26))
        osbp = ctx.enter_context(tc.tile_pool(name="osbp", bufs=6))
        otTp = ctx.enter_context(tc.tile_pool(name="otTp", bufs=8))
        rcpp = ctx.enter_context(tc.tile_pool(name="rcpp", bufs=6))
        foutp = ctx.enter_context(tc.tile_pool(name="foutp", bufs=4))
        psA = ctx.enter_context(tc.tile_pool(name="psA", bufs=2, space="PSUM"))
        psB = ctx.enter_context(tc.tile_pool(name="psB", bufs=4, space="PSUM"))

        # --- load constants ---
        wqk_sb = singles.tile([128, 6, 12, 128], BF16, tag="wqk")
        nc.sync.dma_start(out=wqk_sb, in_=wqk_ap)
        wv_sb = singles.tile([128, 6, 768], BF16, tag="wv")
        nc.sync.dma_start(out=wv_sb, in_=wv_ap)
        wp_sb = singles.tile([128, 6, 768], BF16, tag="wp")
        nc.sync.dma_start(out=wp_sb, in_=wp_ap)
        bq_sb = singles.tile([128, 6], F32, tag="bq")
        nc.sync.dma_start(out=bq_sb, in_=bq_ap)
        bp_sb = singles.tile([1, 768], BF16, tag="bp")
        nc.sync.dma_start(out=bp_sb, in_=bp_ap)
        rt_sb = singles.tile([128, 3, 8, 64], BF16, tag="rt")
        nc.sync.dma_start(out=rt_sb, in_=rt_ap)
        # persistent zero-padded K=128 k-operands, one per head; rows outside
        # the head's 64 channels stay zero forever (memset once here)
        kpad = []
        for h in range(12):
            t = singles.tile([128, N], BF16, tag=f"kpad{h}")
            nc.vector.memset(t, 0.0)
            kpad.append(t)
        g96_sb = singles.tile([128, N], BF16, tag="g96")
        nc.sync.dma_start(out=g96_sb, in_=g96_ap)
        id_sb = singles.tile([128, 128], BF16, tag="ident")
        nc.sync.dma_start(out=id_sb, in_=id_ap)
        ones1 = singles.tile([1, 128], BF16, tag="ones1")
        nc.vector.memset(ones1, 1.0)

        for w in range(WPC):
            xsb = xp.tile([128, 6, N], BF16, tag="x")
            nc.sync.dma_start(out=xsb, in_=xT_ap[w])

            # --- QKV projection (q tiles 0-5, k tiles 6-11; head-pair cols) ---
            qksb = []
            for ot in range(12):
                ps = psB.tile([128, N], F32, tag="ps1")
                for ct in range(6):
                    nc.tensor.matmul(
                        ps,
                        lhsT=wqk_sb[:, ct, ot, :],
                        rhs=xsb[:, ct, :],
                        start=(ct == 0),
                        stop=(ct == 5),
                    )
                if ot < 6:
                    t = qkp.tile([128, N], BF16, tag="qk")
                    if ot % 2 == 0:
                        nc.vector.tensor_scalar_add(
                            out=t, in0=ps, scalar1=bq_sb[:, ot : ot + 1]
                        )
                    else:
                        nc.scalar.activation(
                            out=t, in_=ps, func=AF.Identity,
                            bias=bq_sb[:, ot : ot + 1],
                        )
                    qksb.append(t)
                else:
                    hA, hB = 2 * (ot - 6), 2 * (ot - 6) + 1
                    if ot % 2 == 0:
                        nc.vector.tensor_copy(out=kpad[hA][0:64], in_=ps[0:64])
                        nc.scalar.copy(out=kpad[hB][64:128], in_=ps[64:128])
                    else:
                        nc.scalar.copy(out=kpad[hA][0:64], in_=ps[0:64])
                        nc.vector.tensor_copy(out=kpad[hB][64:128], in_=ps[64:128])

            # --- V (token-major) with ones column per head ---
            vhat = []
            for mt in range(4):
                vt = vhp.tile([128, NH * 65], BF16, tag="vh")
                vt3 = vt.rearrange("p (h e) -> p h e", e=65)
                nc.vector.memset(vt3[:, :, 64], 1.0)
                for half in range(2):
                    ps = psB.tile([128, 384], F32, tag="ps1")
                    for ct in range(6):
                        nc.tensor.matmul(
                            ps,
                            lhsT=xsb[:, ct, mt * 128 : (mt + 1) * 128],
                            rhs=wv_sb[:, ct, 384 * half : 384 * (half + 1)],
                            start=(ct == 0),
                            stop=(ct == 5),
                        )
                    nc.vector.tensor_copy(
                        out=vt3[:, 6 * half : 6 * (half + 1), 0:64],
                        in_=ps.rearrange("p (h c) -> p h c", c=64),
                    )
                vhat.append(vt)

            # --- attention: rel bias, logits^T, exp ---
            pts = []
            for hp in range(6):
                qt = qksb[hp]
                qt4 = qt.rearrange("p (u v z) -> p u v z", v=8, z=8)

                # rel, pair-batched: per axis one (64,512) PSUM; rows 0-31 =
                # head 2hp, rows 32-63 = head 2hp+1 (block-diag rt tables).
                # h-axis stored in std col order; d stored (z,u,v)-major;
                # w stored (v,u,z)-major; un-permuted during the casts.
                rpsa = []
                for a in range(3):
                    rps = psB.tile([64, N], F32, tag="ps1")
                    for j in range(8):
                        if a == 0:  # h: coord = n // 64; dense chunk
                            rhs = qt[:, 64 * j : 64 * (j + 1)]
                        elif a == 1:  # d: coord = n % 8; stream (u, v)
                            rhs = qt4[:, :, :, j]
                        else:  # w: coord = (n // 8) % 8; stream (u, z)
                            rhs = qt4[:, :, j, :]
                        nc.tensor.matmul(
                            rps[:, 64 * j : 64 * (j + 1)],
                            lhsT=rt_sb[:, a, j, :], rhs=rhs,
                            start=(j == 0), stop=(j == 7),
                            skip_group_check=True,
                        )
                    rpsa.append(rps)

                for v in range(2):
                    h = 2 * hp + v
                    s = 32 * v
                    rstage = relp.tile([128, N], BF16, tag="rel")
                    # stored->token un-permutes (token n = 64u + 8v + z)
                    rps_d = rpsa[1][s : s + 32].rearrange(
                        "p (z u v) -> p u v z", u=8, v=8
                    )
                    rps_w = rpsa[2][s : s + 32].rearrange(
                        "p (v u z) -> p u v z", u=8, z=8
                    )
                    e1, e2 = (
                        (nc.vector, nc.scalar) if v == 0 else (nc.scalar, nc.vector)
                    )
                    c1 = e1.tensor_copy if e1 is nc.vector else e1.copy
                    c2 = e2.tensor_copy if e2 is nc.vector else e2.copy
                    c1(out=rstage[0:32], in_=rpsa[0][s : s + 32])
                    c2(out=rstage[32:64], in_=rps_d)
                    c1(out=rstage[64:96], in_=rps_w)
                    nc.gpsimd.memset(rstage[96:128], 0.0)

                    pth = []
                    for mtp in range(2):
                        lps = psA.tile([128, 1024], F32, tag="ps2")
                        for i in range(2):
                            mt = 2 * mtp + i
                            sl = lps[:, 512 * i : 512 * (i + 1)]
                            nc.tensor.matmul(
                                sl,
                                lhsT=kpad[h][:, mt * 128 : (mt + 1) * 128],
                                rhs=qt,
                                start=True,
                                stop=False,
                            )
                            nc.tensor.matmul(
                                sl,
                                lhsT=g96_sb[:, mt * 128 : (mt + 1) * 128],
                                rhs=rstage,
                                start=False,
                                stop=True,
                            )
                        pt_t = ptp.tile([128, 1024], BF16, tag="pt")
                        nc.scalar.activation(out=pt_t, in_=lps, func=AF.Exp)
                        pth.append(pt_t)
                    pts.append(pth)

            # --- PV + 1/s scaling, per token tile ---
            outsb = []
            for nt in range(4):
                halves = []
                for half in range(2):
                    pv = psB.tile([128, 6 * 65], F32, tag="ps1")
                    pv3 = pv.rearrange("p (h e) -> p h e", e=65)
                    for hh in range(6):
                        h = 6 * half + hh
                        for kt_i in range(4):
                            mtp, i = divmod(kt_i, 2)
                            lhsT = pts[h][mtp][
                                :, 512 * i + 128 * nt : 512 * i + 128 * (nt + 1)
                            ]
                            rhs = vhat[kt_i].rearrange("p (g e) -> p g e", e=65)[
                                :, h, :
                            ]
                            nc.tensor.matmul(
                                pv3[:, hh, :], lhsT=lhsT, rhs=rhs,
                                start=(kt_i == 0 and hh == 0),
                                stop=(kt_i == 3),
                                skip_group_check=True,
                            )
                    halves.append(pv)
                rc = rcpp.tile([128, 12], F32, tag="rcp")
                for half in range(2):
                    pv3 = halves[half].rearrange("p (h e) -> p h e", e=65)
                    nc.vector.reciprocal(
                        out=rc[:, 6 * half : 6 * (half + 1)], in_=pv3[:, :, 64]
                    )
                ot_sb = osbp.tile([128, 768], BF16, tag="osb")
                ot3 = ot_sb.rearrange("p (h c) -> p h c", c=64)
                for half in range(2):
                    pv3 = halves[half].rearrange("p (h e) -> p h e", e=65)
                    nc.vector.tensor_mul(
                        out=ot3[:, 6 * half : 6 * (half + 1), :],
                        in0=pv3[:, :, 0:64],
                        in1=_bcast(rc[:, 6 * half : 6 * (half + 1)], 64),
                    )
                outsb.append(ot_sb)

            # --- transpose to channel-major ---
            outT = []
            for ct in range(6):
                tp = psB.tile([128, N], BF16, tag="ps1")
                for nt in range(4):
                    nc.tensor.transpose(
                        out=tp[:, 128 * nt : 128 * (nt + 1)],
                        in_=outsb[nt][:, 128 * ct : 128 * (ct + 1)],
                        identity=id_sb,
                    )
                tT = otTp.tile([128, N], BF16, tag="otT")
                nc.vector.tensor_copy(out=tT, in_=tp)
                outT.append(tT)

            # --- proj + bias, DMA out (384-col halves in separate banks) ---
            for nt in range(4):
                pr = psA.tile([128, 1024], F32, tag="ps2")
                for half in range(2):
                    sl = pr[:, 512 * half : 512 * half + 384]
                    for ct in range(6):
                        nc.tensor.matmul(
                            sl,
                            lhsT=outT[ct][:, 128 * nt : 128 * (nt + 1)],
                            rhs=wp_sb[:, ct, 384 * half : 384 * (half + 1)],
                            start=(ct == 0),
                            stop=False,
                        )
                    nc.tensor.matmul(
                        sl,
                        lhsT=ones1,
                        rhs=bp_sb[:, 384 * half : 384 * (half + 1)],
                        start=False,
                        stop=True,
                    )
                fo = foutp.tile([128, 768], F32, tag="fo")
                pr_v = pr.rearrange("p (h c) -> p h c", c=512)[:, :, 0:384]
                fo_v = fo.rearrange("p (h c) -> p h c", c=384)
                if nt % 2 == 0:
                    nc.vector.tensor_copy(out=fo_v, in_=pr_v)
                else:
                    nc.scalar.copy(out=fo_v, in_=pr_v)
                nc.sync.dma_start(
                    out=out_ap[w, 128 * nt : 128 * (nt + 1), :], in_=fo,
                )


_CACHED = None


def build_module():
    global _CACHED
    if _CACHED is not None:
        return _CACHED
    nc = bacc.Bacc("TRN2", target_bir_lowering=False, debug=False,
                   num_devices=NCORES)
    xT = nc.dram_tensor("xT", [WPC, 128, 6, N], BF16, kind="ExternalInput").ap()
    wqk = nc.dram_tensor("wqk", [128, 6, 12, 128], BF16, kind="ExternalInput").ap()
    wv = nc.dram_tensor("wv", [128, 6, 768], BF16, kind="ExternalInput").ap()
    wp = nc.dram_tensor("wp", [128, 6, 768], BF16, kind="ExternalInput").ap()
    bq = nc.dram_tensor("bq", [128, 6], F32, kind="ExternalInput").ap()
    bp = nc.dram_tensor("bp", [1, 768], BF16, kind="ExternalInput").ap()
    rt = nc.dram_tensor("rt", [128, 3, 8, 64], BF16, kind="ExternalInput").ap()
    g96 = nc.dram_tensor("g96", [128, N], BF16, kind="ExternalInput").ap()
    idm = nc.dram_tensor("idm", [128, 128], BF16, kind="ExternalInput").ap()
    out = nc.dram_tensor("out", [WPC, N, C], F32, kind="ExternalOutput").ap()
    with tile.TileContext(nc) as tc:
        _body(tc, out, xT, wqk, wv, wp, bq, bp, rt, g96, idm)
    nc.compile()
    _CACHED = nc
    return nc


def prep_inputs(inputs):
    """Host-side layout/folding. Returns the shared in_map (no xT) and the
    per-core xT arrays."""
    x = np.asarray(inputs["x"], np.float32)
    qkv_w = np.asarray(inputs["qkv_w"], np.float32)
    qkv_b = np.asarray(inputs["qkv_b"], np.float32)
    proj_w = np.asarray(inputs["proj_w"], np.float32)
    proj_b = np.asarray(inputs["proj_b"], np.float32)
    lr = float(np.asarray(inputs["lr"]))
    scale = HD ** -0.5

    Wq, Wk, Wv = qkv_w[0:C], qkv_w[C : 2 * C], qkv_w[2 * C : 3 * C]
    bq_, bv_ = qkv_b[0:C], qkv_b[2 * C : 3 * C]

    # wqk: [p, ct, ot, mc]; o-tiles 0-5 = q head-pairs, 6-11 = scale*Wk pairs
    wqk = np.zeros((128, 6, 12, 128), np.float32)
    for ot in range(12):
        for half in range(2):
            if ot < 6:
                head = 2 * ot + half
                Wsrc = Wq[head * 64 : (head + 1) * 64]
            else:
                head = 2 * (ot - 6) + half
                Wsrc = Wk[head * 64 : (head + 1) * 64] * scale
            wt = Wsrc.T.reshape(6, 128, 64).transpose(1, 0, 2)  # (p, ct, 64)
            wqk[:, :, ot, 64 * half : 64 * (half + 1)] = wt

    wv = Wv.T.reshape(6, 128, C).transpose(1, 0, 2)  # (p, ct, co)
    wp = proj_w.T.reshape(6, 128, C).transpose(1, 0, 2)
    bq_t = bq_.reshape(6, 128).T.copy()  # (128, 6), head-pair order matches
    bp1 = (proj_b + bv_ @ proj_w.T).reshape(1, C)

    # region order: h, d, w (matches the on-device rel layout)
    tabs = [
        _rel_table(np.asarray(inputs["rel_pos_h"], np.float32), 8) * lr,
        _rel_table(np.asarray(inputs["rel_pos_d"], np.float32), 8) * lr,
        _rel_table(np.asarray(inputs["rel_pos_w"], np.float32), 8) * lr,
    ]
    # rt: block-diag pair tables — lhsT cols 0-31 give head A's 32-row rel
    # block (contracting its q rows 0-63), cols 32-63 head B's (rows 64-127)
    rt = np.zeros((128, 3, 8, 64), np.float32)
    for a in range(3):
        t = tabs[a].transpose(2, 0, 1)  # (64c, 8j, 8k)
        rt[0:64, a, :, 0:8] = t
        rt[64:128, a, :, 32:40] = t

    m = np.arange(N)
    g96 = np.zeros((128, N), np.float32)
    coords = [m // 64, m % 8, (m // 8) % 8]
    for a in range(3):
        for k in range(8):
            g96[32 * a + k] = coords[a] == k

    xall = (
        x.reshape(B, N, 6, 128).transpose(0, 3, 2, 1).astype(BF)
    )  # (B, p, ct, n)

    shared = {
        "wqk": wqk.astype(BF),
        "wv": wv.astype(BF),
        "wp": wp.astype(BF),
        "bq": bq_t.astype(np.float32),
        "bp": bp1.astype(BF),
        "rt": rt.astype(BF),
        "g96": g96.astype(BF),
        "idm": np.eye(128, dtype=np.float32).astype(BF),
    }
    xT_cores = [xall[WPC * i : WPC * (i + 1)] for i in range(NCORES)]
    return shared, xT_cores


def assemble_output(results):
    outs = [np.asarray(r["out"], np.float32) for r in results]
    full = np.concatenate(outs, axis=0)  # (32, 512, 768)
    return full.reshape(B, HH, WW, DD, C)


def kernel(x, qkv_w, qkv_b, proj_w, proj_b, rel_pos_h, rel_pos_w, rel_pos_d, lr,
           _trace=False):
    nc = build_module()
    shared, xT_cores = prep_inputs(dict(
        x=x, qkv_w=qkv_w, qkv_b=qkv_b, proj_w=proj_w, proj_b=proj_b,
        rel_pos_h=rel_pos_h, rel_pos_w=rel_pos_w, rel_pos_d=rel_pos_d, lr=lr,
    ))
    in_maps = [{**shared, "xT": xT_cores[i]} for i in range(NCORES)]
    res = run_bass_kernel_spmd(nc, in_maps, list(range(NCORES)), trace=_trace)
    out = assemble_output(res.results)
    if _trace:
        kernel.last_exec_time_ns = res.exec_time_ns
        kernel.last_profile = res
    return out


# revision 59
# speedup vs baseline: 1.1206x; 1.1088x over previous
"""nn_Attention_3d — 3D windowed attention with decomposed relative position
biases, on 8 Trainium2 NeuronCores via Bass/Tile.

Sharding: data-parallel over the window dim B (32 windows -> 4 per core).
Weights / rel-pos tables replicated.

Math notes (vs reference):
  - softmax max-subtraction dropped: logits are O(1) (inputs ~N(0,1), weights
    ~0.02 scale), exp cannot overflow; softmax is shift-invariant so the
    result is identical.
  - q-side qkv bias kept (it feeds the rel-pos terms and the k-dot); k-side
    bias dropped (adds a per-row constant to logits -> softmax invariant).
  - v-side bias folded into the proj bias on host: (o + bv) @ Wp^T + bp
    = o @ Wp^T + (bv @ Wp^T + bp).
  - attn scale folded into Wk on host; lr folded into the rel tables.

Device pipeline per window (N=512 tokens, C=768, 12 heads, hd=64):
  qkvT   : per-head-pair column-reordered QKV matmul, bf16, PSUM->SBUF evac
  v      : token-major V matmul with an appended ones column per head
           (the PV matmul then yields the softmax denominator for free)
  rel    : per (head, axis, coord-chunk) tiny matmuls against host-built
           rel tables -> relsel^T (24 x 512) + zero padding to 96 rows
  logits : logits^T[m,n] = k^T.T @ q^T  (+ G96.T @ relstage accumulate)
  exp    : ScalarE Exp, PSUM fp32 -> SBUF bf16 (p^T tiles)
  PV     : out[n, head*65+c] accumulated over m K-tiles; col 64 = sum = s[n]
  scale  : out *= 1/s per (row, head) on VectorE
  trans  : TensorE transpose to channel-major for the proj matmul
  proj   : out @ Wp^T + bias via K=1 ones-row matmul; PSUM -> DRAM DMA
"""

import sys

sys.path.insert(0, "/opt/trn_rl_repo")

import numpy as np
import ml_dtypes

import concourse.bass as bass
import concourse.tile as tile
from concourse import bacc, mybir
from concourse.bass_utils import run_bass_kernel_spmd

F32 = mybir.dt.float32
BF16 = mybir.dt.bfloat16
F8 = mybir.dt.float8e4
AF = mybir.ActivationFunctionType
DR = mybir.MatmulPerfMode.DoubleRow

B, HH, WW, DD, C = 32, 8, 8, 8, 768
NH, HD = 12, 64
N = HH * WW * DD  # 512
NCORES = 8
WPC = B // NCORES  # 4 windows per core
BF = ml_dtypes.bfloat16
F8NP = ml_dtypes.float8_e4m3


def _rel_table(rel_pos, n):
    idx = np.arange(n)[:, None] - np.arange(n)[None, :] + (n - 1)
    return rel_pos[idx]  # (n, n, hd)


def _bcast(ap, n):
    """Append a stride-0 free dim of size n to an AP."""
    return bass.AP(tensor=ap.tensor, offset=ap.offset, ap=[*ap.ap, [0, n]])


def _body(tc, out_ap, xT_ap, wqk_ap, wv_ap, wp_ap, bq_ap, bp_ap, rt_ap, g96_ap, id_ap):
    nc = tc.nc
    import contextlib

    ctx = contextlib.ExitStack()
    with ctx:
        singles = ctx.enter_context(tc.tile_pool(name="singles", bufs=1))
        xp = ctx.enter_context(tc.tile_pool(name="xp", bufs=2))
        qkp = ctx.enter_context(tc.tile_pool(name="qkp", bufs=8))
        vhp = ctx.enter_context(tc.tile_pool(name="vhp", bufs=6))
        ptp = ctx.enter_context(tc.tile_pool(name="ptp", bufs=26))
        osbp = ctx.enter_context(tc.tile_pool(name="osbp", bufs=6))
        otTp = ctx.enter_context(tc.tile_pool(name="otTp", bufs=8))
        rcpp = ctx.enter_context(tc.tile_pool(name="rcpp", bufs=6))
        foutp = ctx.enter_context(tc.tile_pool(name="foutp", bufs=4))
        psA = ctx.enter_context(tc.tile_pool(name="psA", bufs=2, space="PSUM"))
        psB = ctx.enter_context(tc.tile_pool(name="psB", bufs=4, space="PSUM"))

        # --- load constants ---
        wqk_sb = singles.tile([128, 6, 12, 128], BF16, tag="wqk")
        nc.sync.dma_start(out=wqk_sb, in_=wqk_ap)
        wv_sb = singles.tile([128, 6, 768], BF16, tag="wv")
        nc.sync.dma_start(out=wv_sb, in_=wv_ap)
        wp_sb = singles.tile([128, 6, 768], BF16, tag="wp")
        nc.sync.dma_start(out=wp_sb, in_=wp_ap)
        bq_sb = singles.tile([128, 6], F32, tag="bq")
        nc.sync.dma_start(out=bq_sb, in_=bq_ap)
        bp_sb = singles.tile([1, 768], BF16, tag="bp")
        nc.sync.dma_start(out=bp_sb, in_=bp_ap)
        rt_sb = singles.tile([128, 3, 8, 64], BF16, tag="rt")
        nc.sync.dma_start(out=rt_sb, in_=rt_ap)
        # persistent fp8 DoubleRow k-operands, one per head: subtile 0 =
        # zero-padded k (rows outside the head's 64 channels stay zero),
        # subtile 1 = the constant rel selector matrix G.
        kg = []
        for h in range(12):
            t = singles.tile([128, 2, N], F8, tag=f"kg{h}")
            nc.vector.memset(t[:, 0, :], 0.0)
            nc.sync.dma_start(out=t[:, 1, :], in_=g96_ap)
            kg.append(t)
        id_sb = singles.tile([128, 128], BF16, tag="ident")
        nc.sync.dma_start(out=id_sb, in_=id_ap)
        ones1 = singles.tile([1, 128], BF16, tag="ones1")
        nc.vector.memset(ones1, 1.0)

        for w in range(WPC):
            xsb = xp.tile([128, 6, N], BF16, tag="x")
            nc.sync.dma_start(out=xsb, in_=xT_ap[w])

            # --- QKV projection (q tiles 0-5, k tiles 6-11; head-pair cols) ---
            qksb = []
            for ot in range(12):
                ps = psB.tile([128, N], F32, tag="ps1")
                for ct in range(6):
                    nc.tensor.matmul(
                        ps,
                        lhsT=wqk_sb[:, ct, ot, :],
                        rhs=xsb[:, ct, :],
                        start=(ct == 0),
                        stop=(ct == 5),
                    )
                if ot < 6:
                    # qrel: subtile 0 = q (pair), subtiles 1/2 = relstage A/B
                    t = qkp.tile([128, 3, N], F8, tag="qk")
                    if ot % 2 == 0:
                        nc.vector.tensor_scalar_add(
                            out=t[:, 0, :], in0=ps, scalar1=bq_sb[:, ot : ot + 1]
                        )
                    else:
                        nc.scalar.activation(
                            out=t[:, 0, :], in_=ps, func=AF.Identity,
                            bias=bq_sb[:, ot : ot + 1],
                        )
                    qksb.append(t)
                else:
                    hA, hB = 2 * (ot - 6), 2 * (ot - 6) + 1
                    if ot % 2 == 0:
                        nc.vector.tensor_copy(out=kg[hA][0:64, 0, :], in_=ps[0:64])
                        nc.scalar.copy(out=kg[hB][64:128, 0, :], in_=ps[64:128])
                    else:
                        nc.scalar.copy(out=kg[hA][0:64, 0, :], in_=ps[0:64])
                        nc.vector.tensor_copy(out=kg[hB][64:128, 0, :], in_=ps[64:128])

            # --- V (token-major) with ones column per head ---
            vhat = []
            for mt in range(4):
                vt = vhp.tile([128, NH * 65], BF16, tag="vh")
                vt3 = vt.rearrange("p (h e) -> p h e", e=65)
                nc.vector.memset(vt3[:, :, 64], 1.0)
                for half in range(2):
                    ps = psB.tile([128, 384], F32, tag="ps1")
                    for ct in range(6):
                        nc.tensor.matmul(
                            ps,
                            lhsT=xsb[:, ct, mt * 128 : (mt + 1) * 128],
                            rhs=wv_sb[:, ct, 384 * half : 384 * (half + 1)],
                            start=(ct == 0),
                            stop=(ct == 5),
                        )
                    nc.vector.tensor_copy(
                        out=vt3[:, 6 * half : 6 * (half + 1), 0:64],
                        in_=ps.rearrange("p (h c) -> p h c", c=64),
                    )
                vhat.append(vt)

            # --- attention: rel bias, logits^T, exp ---
            pts = []
            for hp in range(6):
                qrel = qksb[hp]
                qt = qrel[:, 0, :]
                qt4 = qt.rearrange("p (u v z) -> p u v z", v=8, z=8)

                # rel, pair-batched: per axis one (64,512) PSUM; rows 0-31 =
                # head 2hp, rows 32-63 = head 2hp+1 (block-diag rt tables).
                # h-axis stored in std col order; d stored (z,u,v)-major;
                # w stored (v,u,z)-major; un-permuted during the casts.
                rpsa = []
                for a in range(3):
                    rps = psB.tile([64, N], F32, tag="ps1")
                    for j in range(8):
                        if a == 0:  # h: coord = n // 64; dense chunk
                            rhs = qt[:, 64 * j : 64 * (j + 1)]
                        elif a == 1:  # d: coord = n % 8; stream (u, v)
                            rhs = qt4[:, :, :, j]
                        else:  # w: coord = (n // 8) % 8; stream (u, z)
                            rhs = qt4[:, :, j, :]
                        nc.tensor.matmul(
                            rps[:, 64 * j : 64 * (j + 1)],
                            lhsT=rt_sb[:, a, j, :], rhs=rhs,
                            start=(j == 0), stop=(j == 7),
                            skip_group_check=True,
                        )
                    rpsa.append(rps)

                for v in range(2):
                    h = 2 * hp + v
                    s = 32 * v
                    rstage = qrel[:, 1 + v, :]
                    # stored->token un-permutes (token n = 64u + 8v + z)
                    rps_d = rpsa[1][s : s + 32].rearrange(
                        "p (z u v) -> p u v z", u=8, v=8
                    )
                    rps_w = rpsa[2][s : s + 32].rearrange(
                        "p (v u z) -> p u v z", u=8, z=8
                    )
                    e1, e2 = (
                        (nc.vector, nc.scalar) if v == 0 else (nc.scalar, nc.vector)
                    )
                    c1 = e1.tensor_copy if e1 is nc.vector else e1.copy
                    c2 = e2.tensor_copy if e2 is nc.vector else e2.copy
                    c1(out=rstage[0:32], in_=rpsa[0][s : s + 32])
                    c2(out=rstage[32:64], in_=rps_d)
                    c1(out=rstage[64:96], in_=rps_w)
                    nc.gpsimd.memset(rstage[96:128], 0.0)

                    # rhs subtiles {0, 1+v} of qrel: (q, relstage_h)
                    qr2 = bass.AP(
                        tensor=qrel.tensor,
                        offset=qrel[:].offset,
                        ap=[qrel[:].ap[0], [(1 + v) * N, 2], [1, N]],
                    )
                    pth = []
                    for mtp in range(2):
                        lps = psA.tile([128, 1024], F32, tag="ps2")
                        for i in range(2):
                            mt = 2 * mtp + i
                            sl = lps[:, 512 * i : 512 * (i + 1)]
                            nc.tensor.matmul(
                                sl,
                                lhsT=kg[h][:, :, mt * 128 : (mt + 1) * 128],
                                rhs=qr2,
                                start=True,
                                stop=True,
                                perf_mode=DR,
                            )
                        pt_t = ptp.tile([128, 1024], BF16, tag="pt")
                        nc.scalar.activation(out=pt_t, in_=lps, func=AF.Exp)
                        pth.append(pt_t)
                    pts.append(pth)

            # --- PV + 1/s scaling, per token tile ---
            outsb = []
            for nt in range(4):
                halves = []
                for half in range(2):
                    pv = psB.tile([128, 6 * 65], F32, tag="ps1")
                    pv3 = pv.rearrange("p (h e) -> p h e", e=65)
                    for hh in range(6):
                        h = 6 * half + hh
                        for kt_i in range(4):
                            mtp, i = divmod(kt_i, 2)
                            lhsT = pts[h][mtp][
                                :, 512 * i + 128 * nt : 512 * i + 128 * (nt + 1)
                            ]
                            rhs = vhat[kt_i].rearrange("p (g e) -> p g e", e=65)[
                                :, h, :
                            ]
                            nc.tensor.matmul(
                                pv3[:, hh, :], lhsT=lhsT, rhs=rhs,
                                start=(kt_i == 0 and hh == 0),
                                stop=(kt_i == 3),
                                skip_group_check=True,
                            )
                    halves.append(pv)
                rc = rcpp.tile([128, 12], F32, tag="rcp")
                for half in range(2):
                    pv3 = halves[half].rearrange("p (h e) -> p h e", e=65)
                    nc.vector.reciprocal(
                        out=rc[:, 6 * half : 6 * (half + 1)], in_=pv3[:, :, 64]
                    )
                ot_sb = osbp.tile([128, 768], BF16, tag="osb")
                ot3 = ot_sb.rearrange("p (h c) -> p h c", c=64)
                for half in range(2):
                    pv3 = halves[half].rearrange("p (h e) -> p h e", e=65)
                    nc.vector.tensor_mul(
                        out=ot3[:, 6 * half : 6 * (half + 1), :],
                        in0=pv3[:, :, 0:64],
                        in1=_bcast(rc[:, 6 * half : 6 * (half + 1)], 64),
                    )
                outsb.append(ot_sb)

            # --- transpose to channel-major (fp8 for DoubleRow proj) ---
            outT = otTp.tile([128, 6, N], BF16, tag="otT")
            for ct in range(6):
                tp = psB.tile([128, N], BF16, tag="ps1")
                for nt in range(4):
                    nc.tensor.transpose(
                        out=tp[:, 128 * nt : 128 * (nt + 1)],
                        in_=outsb[nt][:, 128 * ct : 128 * (ct + 1)],
                        identity=id_sb,
                    )
                nc.vector.tensor_copy(out=outT[:, ct, :], in_=tp)

            # --- proj + bias, DMA out (384-col halves in separate banks) ---
            for nt in range(4):
                pr = psA.tile([128, 1024], F32, tag="ps2")
                for half in range(2):
                    sl = pr[:, 512 * half : 512 * half + 384]
                    for ct in range(6):
                        nc.tensor.matmul(
                            sl,
                            lhsT=outT[:, ct, 128 * nt : 128 * (nt + 1)],
                            rhs=wp_sb[:, ct, 384 * half : 384 * (half + 1)],
                            start=(ct == 0),
                            stop=False,
                        )
                    nc.tensor.matmul(
                        sl,
                        lhsT=ones1,
                        rhs=bp_sb[:, 384 * half : 384 * (half + 1)],
                        start=False,
                        stop=True,
                    )
                fo = foutp.tile([128, 768], F32, tag="fo")
                pr_v = pr.rearrange("p (h c) -> p h c", c=512)[:, :, 0:384]
                fo_v = fo.rearrange("p (h c) -> p h c", c=384)
                if nt % 2 == 0:
                    nc.vector.tensor_copy(out=fo_v, in_=pr_v)
                else:
                    nc.scalar.copy(out=fo_v, in_=pr_v)
                nc.sync.dma_start(
                    out=out_ap[w, 128 * nt : 128 * (nt + 1), :], in_=fo,
                )


_CACHED = None


def build_module():
    global _CACHED
    if _CACHED is not None:
        return _CACHED
    nc = bacc.Bacc("TRN2", target_bir_lowering=False, debug=False,
                   num_devices=NCORES)
    xT = nc.dram_tensor("xT", [WPC, 128, 6, N], BF16, kind="ExternalInput").ap()
    wqk = nc.dram_tensor("wqk", [128, 6, 12, 128], BF16, kind="ExternalInput").ap()
    wv = nc.dram_tensor("wv", [128, 6, 768], BF16, kind="ExternalInput").ap()
    wp = nc.dram_tensor("wp", [128, 6, 768], BF16, kind="ExternalInput").ap()
    bq = nc.dram_tensor("bq", [128, 6], F32, kind="ExternalInput").ap()
    bp = nc.dram_tensor("bp", [1, 768], BF16, kind="ExternalInput").ap()
    rt = nc.dram_tensor("rt", [128, 3, 8, 64], BF16, kind="ExternalInput").ap()
    g96 = nc.dram_tensor("g96", [128, N], F8, kind="ExternalInput").ap()
    idm = nc.dram_tensor("idm", [128, 128], BF16, kind="ExternalInput").ap()
    out = nc.dram_tensor("out", [WPC, N, C], F32, kind="ExternalOutput").ap()
    with tile.TileContext(nc) as tc:
        _body(tc, out, xT, wqk, wv, wp, bq, bp, rt, g96, idm)
    nc.compile()
    _CACHED = nc
    return nc


def prep_inputs(inputs):
    """Host-side layout/folding. Returns the shared in_map (no xT) and the
    per-core xT arrays."""
    x = np.asarray(inputs["x"], np.float32)
    qkv_w = np.asarray(inputs["qkv_w"], np.float32)
    qkv_b = np.asarray(inputs["qkv_b"], np.float32)
    proj_w = np.asarray(inputs["proj_w"], np.float32)
    proj_b = np.asarray(inputs["proj_b"], np.float32)
    lr = float(np.asarray(inputs["lr"]))
    scale = HD ** -0.5

    Wq, Wk, Wv = qkv_w[0:C], qkv_w[C : 2 * C], qkv_w[2 * C : 3 * C]
    bq_, bv_ = qkv_b[0:C], qkv_b[2 * C : 3 * C]

    # wqk: [p, ct, ot, mc]; o-tiles 0-5 = q head-pairs, 6-11 = scale*Wk pairs
    wqk = np.zeros((128, 6, 12, 128), np.float32)
    for ot in range(12):
        for half in range(2):
            if ot < 6:
                head = 2 * ot + half
                Wsrc = Wq[head * 64 : (head + 1) * 64]
            else:
                head = 2 * (ot - 6) + half
                Wsrc = Wk[head * 64 : (head + 1) * 64] * scale
            wt = Wsrc.T.reshape(6, 128, 64).transpose(1, 0, 2)  # (p, ct, 64)
            wqk[:, :, ot, 64 * half : 64 * (half + 1)] = wt

    wv = Wv.T.reshape(6, 128, C).transpose(1, 0, 2)  # (p, ct, co)
    wp = proj_w.T.reshape(6, 128, C).transpose(1, 0, 2)
    bq_t = bq_.reshape(6, 128).T.copy()  # (128, 6), head-pair order matches
    bp1 = (proj_b + bv_ @ proj_w.T).reshape(1, C)

    # region order: h, d, w (matches the on-device rel layout)
    tabs = [
        _rel_table(np.asarray(inputs["rel_pos_h"], np.float32), 8) * lr,
        _rel_table(np.asarray(inputs["rel_pos_d"], np.float32), 8) * lr,
        _rel_table(np.asarray(inputs["rel_pos_w"], np.float32), 8) * lr,
    ]
    # rt: block-diag pair tables — lhsT cols 0-31 give head A's 32-row rel
    # block (contracting its q rows 0-63), cols 32-63 head B's (rows 64-127)
    rt = np.zeros((128, 3, 8, 64), np.float32)
    for a in range(3):
        t = tabs[a].transpose(2, 0, 1)  # (64c, 8j, 8k)
        rt[0:64, a, :, 0:8] = t
        rt[64:128, a, :, 32:40] = t

    m = np.arange(N)
    g96 = np.zeros((128, N), np.float32)
    coords = [m // 64, m % 8, (m // 8) % 8]
    for a in range(3):
        for k in range(8):
            g96[32 * a + k] = coords[a] == k

    xall = (
        x.reshape(B, N, 6, 128).transpose(0, 3, 2, 1).astype(BF)
    )  # (B, p, ct, n)

    shared = {
        "wqk": wqk.astype(BF),
        "wv": wv.astype(BF),
        "wp": wp.astype(BF),
        "bq": bq_t.astype(np.float32),
        "bp": bp1.astype(BF),
        "rt": rt.astype(BF),
        "g96": g96.astype(F8NP),
        "idm": np.eye(128, dtype=np.float32).astype(BF),
    }
    xT_cores = [xall[WPC * i : WPC * (i + 1)] for i in range(NCORES)]
    return shared, xT_cores


def assemble_output(results):
    outs = [np.asarray(r["out"], np.float32) for r in results]
    full = np.concatenate(outs, axis=0)  # (32, 512, 768)
    return full.reshape(B, HH, WW, DD, C)


def kernel(x, qkv_w, qkv_b, proj_w, proj_b, rel_pos_h, rel_pos_w, rel_pos_d, lr,
           _trace=False):
    nc = build_module()
    shared, xT_cores = prep_inputs(dict(
        x=x, qkv_w=qkv_w, qkv_b=qkv_b, proj_w=proj_w, proj_b=proj_b,
        rel_pos_h=rel_pos_h, rel_pos_w=rel_pos_w, rel_pos_d=rel_pos_d, lr=lr,
    ))
    in_maps = [{**shared, "xT": xT_cores[i]} for i in range(NCORES)]
    res = run_bass_kernel_spmd(nc, in_maps, list(range(NCORES)), trace=_trace)
    out = assemble_output(res.results)
    if _trace:
        kernel.last_exec_time_ns = res.exec_time_ns
        kernel.last_profile = res
    return out
